# revision 37
# baseline (speedup 1.0000x reference)
"""Trainium2 Bass kernel for nn_ChebNet (complex Chebyshev GNN layer).

Sharding: data-parallel over batch B=8 across the 8 NeuronCores (one batch
element per core). No inter-core communication.

v3 design (vs 249us v2): full phase overlap + engine balance.

Structure (per core, batch b; N=1024, C=OC=64, K+1=5, j-chunks CH=8):
  era1: DMA weights/X; PE transposes X^T, attention projections, src
    broadcast rows, dst cols, A/B output stacks (bf16). L (bf16,
    host-pretiled [k][c][j][t][i]) prefetch starts at t=0.
  era2: attention per j-chunk in TRANSPOSED layout [j, i] (softmax over i
    = free axis): ACT chain Prelu x2 -> (Pool squares, Pool m2-add) ->
    Ln -> Exp(0.5 ln) = mag -> Exp(mag)+accum colsum -> er = Exp(mag -
    0.5 lnm2); DVE: d-stt, ar/ai stt (bf16), mai. Ln/mag/er ops merged
    over chunk PAIRS. One act table set (natural_log_exp) enforced by a
    post-compile fixup (the stock pass thrashes 0<->5 per chunk).
    Meanwhile PE runs diag-product passes k0,k1 gated per chunk.
  era3: PE passes k2..k4 at full clock. Per k-pass PSUM [128, 8ib, 2t,
    128] (4 banks, 2 passes in flight): per (c, ib) 3 matmuls:
    ar x (Lr|Li) -> (t0,t1); mai x Li -> t0; ai x Lr -> t1, so psum
    t0 = sum_j ar*Lr - ai*Li (= SLr diag block) and t1 = SLi. One
    start=True per bank (c==0, even ib) resets it; no zero-matmuls.
    Extraction per pass: ACT copy psum->bf16, DVE mask-mult (2x bf16),
    DVE reduce -> slAll[128, ib, t, k] strided. Tail per ib: Pool/DVE
    broadcast-tt Z = SL (x) A/B stacks -> strided reduce -> out DMA.
"""

import numpy as np
from contextlib import ExitStack

B, N, C, OC = 8, 1024, 64, 64
K1 = 5
P = 128
CH = N // P
NCORES = 8
EPS = 1e-12

_CACHE = {}


def _build_nc():
    import concourse.tile as tile
    from concourse import bacc, mybir

    f32 = mybir.dt.float32
    bf16 = mybir.dt.bfloat16
    Alu = mybir.AluOpType
    Act = mybir.ActivationFunctionType

    nc = bacc.Bacc("TRN2", target_bir_lowering=False, debug=False,
                   enable_asserts=False, num_devices=NCORES)

    # host-pretiled to [p 128][c 8][d 64] so the DMA is contiguous
    x_real = nc.dram_tensor("x_real", [P, CH, C], f32, kind="ExternalInput").ap()
    x_imag = nc.dram_tensor("x_imag", [P, CH, C], f32, kind="ExternalInput").ap()
    # host-pretiled: [k][c][j 128][t 2][i 1024], t0=Lr^T, t1=Li^T (bf16)
    lcat = nc.dram_tensor("lcat", [K1, CH, P, 2, N], bf16,
                          kind="ExternalInput").ap()
    w4 = nc.dram_tensor("w4", [2 * C, 4], f32, kind="ExternalInput").ap()
    wcat = nc.dram_tensor("wcat", [2 * C, K1 * OC], bf16,
                          kind="ExternalInput").ap()
    pa_cols = nc.dram_tensor("pa_cols", [P, 2], f32, kind="ExternalInput").ap()
    ab2 = nc.dram_tensor("ab2", [1, 2], f32, kind="ExternalInput").ap()
    out_r = nc.dram_tensor("out_r", [N, OC], f32, kind="ExternalOutput").ap()
    out_i = nc.dram_tensor("out_i", [N, OC], f32, kind="ExternalOutput").ap()

    with tile.TileContext(nc) as tc:
        with ExitStack() as ctx:
            _emit(ctx, tc, nc, mybir, f32, bf16, Alu, Act,
                  x_real, x_imag, lcat, w4, wcat, pa_cols, ab2, out_r, out_i)
    nc.compile()

    # --- act-table fixup: the stock placement alternates between table
    # sets 0 (exp) and 5 (ln) every chunk (~1.3us per reload). Set 6
    # (natural_log_exp_and_others) serves every function used here
    # (parametric_relu, square, ln, exp, copy), so keep one load of it.
    nloads = 0
    for b in nc.main_func.blocks:
        keep = []
        for inst in b.instructions:
            if isinstance(inst, mybir.InstLoadActFuncSet):
                nloads += 1
                if nloads == 1:
                    inst.act_func_set_id = 6
                    keep.append(inst)
                continue
            keep.append(inst)
        b.instructions[:] = keep
    return nc


def _emit(ctx, tc, nc, mybir, f32, bf16, Alu, Act,
          x_real, x_imag, lcat, w4, wcat, pa_cols, ab2, out_r, out_i):
    X = mybir.AxisListType.X

    const = ctx.enter_context(tc.tile_pool(name="const", bufs=1))
    persist = ctx.enter_context(tc.tile_pool(name="persist", bufs=1))

    ident_i = const.tile([P, P], mybir.dt.int32)
    nc.gpsimd.iota(ident_i[:], pattern=[[1, P]], base=0, channel_multiplier=-1)
    ident = const.tile([P, P], f32)
    nc.vector.tensor_scalar(ident[:], ident_i[:], 0, None, op0=Alu.is_equal)
    mask_bf = const.tile([P, P], bf16)
    nc.vector.tensor_scalar(mask_bf[:], ident_i[:], 0, None, op0=Alu.is_equal)

    ones_row = const.tile([1, P], f32)
    nc.vector.memset(ones_row[:], 1.0)
    eps_col = const.tile([P, 1], f32)
    nc.vector.memset(eps_col[:], EPS)

    w4_sb = const.tile([P, 4], f32)
    nc.sync.dma_start(w4_sb[:], w4[:])
    wcat_sb = const.tile([P, K1 * OC], bf16)
    nc.sync.dma_start(wcat_sb[:], wcat[:])
    pa_sb = const.tile([P, 2], f32)
    nc.sync.dma_start(pa_sb[:], pa_cols[:])
    ab_sb = const.tile([1, 2], f32)
    nc.sync.dma_start(ab_sb[:], ab2[:])

    # ---------------- persistent tiles ----------------
    src_bc_r = persist.tile([P, N], f32)
    src_bc_i = persist.tile([P, N], f32)
    dstT = persist.tile([P, 2 * CH], f32)

    a_pool = ctx.enter_context(tc.tile_pool(name="apool", bufs=3 * CH))
    ar_t, ai_t, mai_t = [], [], []
    for c in range(CH):
        ar_t.append(a_pool.tile([P, N], bf16, tag="apool", name=f"ar{c}"))
        ai_t.append(a_pool.tile([P, N], bf16, tag="apool", name=f"ai{c}"))
        mai_t.append(a_pool.tile([P, N], bf16, tag="apool", name=f"mai{c}"))

    ab_stack = ctx.enter_context(tc.tile_pool(name="abst", bufs=2 * CH))
    As_t, Bs_t = [], []
    for c in range(CH):
        As_t.append(ab_stack.tile([P, K1, OC], bf16, tag="abst", name=f"As{c}"))
        Bs_t.append(ab_stack.tile([P, K1, OC], bf16, tag="abst", name=f"Bs{c}"))

    # L tiles: rotating pool, one tile per (k, c) = [128j, 2t, 1024i] bf16
    # (DMAs emitted after the phase-A loads so X isn't queued behind 20MB)
    l_pool = ctx.enter_context(tc.tile_pool(name="lpool", bufs=10))
    l_tiles = {}

    # SL results [p, ib, t(R/I), k] fp32
    slAll = persist.tile([P, CH, 2, K1], f32)
    cs_cols = persist.tile([P, CH], f32)     # per-chunk softmax colsums
    inv_cols = persist.tile([P, CH], f32)

    # ---------------- era 1: transposes / projections / stacks ----------
    pa_ctx = ExitStack()
    pa_pool = pa_ctx.enter_context(tc.tile_pool(name="phaseA", bufs=1))
    with tc.tile_pool(name="pa_ps", bufs=2, space="PSUM") as pa_ps, \
         tc.tile_pool(name="pa_proj", bufs=3, space="PSUM") as pa_proj:
        xt = pa_pool.tile([P, N], f32)         # [Xr^T; Xi^T]
        xtb = pa_pool.tile([P, N], bf16)
        xr_sb = pa_pool.tile([P, CH, C], f32)
        xi_sb = pa_pool.tile([P, CH, C], f32)
        for q in range(4):
            qs = slice(q * 2, q * 2 + 2)
            nc.sync.dma_start(xr_sb[:, qs, :], x_real[:, qs, :])
            nc.sync.dma_start(xi_sb[:, qs, :], x_imag[:, qs, :])

        # L prefetch: emitted now (after X/weight loads) so it starts at
        # t~0 without delaying the phase-A inputs.
        for k in range(K1):
            for c in range(CH):
                lt = l_pool.tile([P, 2, N], bf16, tag="lpool",
                                 name=f"L{k}_{c}")
                l_tiles[(k, c)] = lt
                nc.sync.dma_start(lt[:], lcat[k, c])

        # all 8 transposes per r/i go into one psum tile -> ONE copy each.
        # tile is 2 banks (1024 f32); start=True resets a whole bank, so
        # flag it only on the first write of each bank (c==0, c==4).
        tpr = pa_ps.tile([C, CH, P], f32, tag="tp")
        for c in range(CH):
            nc.tensor.matmul(tpr[:, c, :], xr_sb[:, c, :], ident[:],
                             is_transpose=True, start=(c in (0, 4)),
                             stop=(c in (3, 7)), skip_group_check=True)
            if c == 3:
                nc.vector.tensor_copy(
                    xt[0:C, 0:512], tpr[:, 0:4, :].rearrange("p a b -> p (a b)"))
        nc.vector.tensor_copy(
            xt[0:C, 512:1024], tpr[:, 4:8, :].rearrange("p a b -> p (a b)"))
        tpi = pa_ps.tile([C, CH, P], f32, tag="tp")
        for c in range(CH):
            nc.tensor.matmul(tpi[:, c, :], xi_sb[:, c, :], ident[:],
                             is_transpose=True, start=(c in (0, 4)),
                             stop=(c in (3, 7)), skip_group_check=True)
            if c == 3:
                nc.vector.tensor_copy(
                    xt[C:2 * C, 0:512],
                    tpi[:, 0:4, :].rearrange("p a b -> p (a b)"))
        nc.vector.tensor_copy(xt[C:2 * C, 512:1024],
                       tpi[:, 4:8, :].rearrange("p a b -> p (a b)"))

        nc.vector.tensor_copy(xtb[:], xt[:])

        srcr_sb = pa_pool.tile([1, N], f32)
        srci_sb = pa_pool.tile([1, N], f32)
        dst_sb = pa_pool.tile([2, N], f32)
        for h in range(2):
            hs = slice(h * 512, (h + 1) * 512)
            srcr_ps = pa_proj.tile([1, 512], f32, tag="proj")
            nc.tensor.matmul(srcr_ps[:], w4_sb[:, 0:1], xt[:, hs],
                             start=True, stop=True)
            nc.vector.tensor_copy(srcr_sb[:, hs], srcr_ps[:])
            srci_ps = pa_proj.tile([1, 512], f32, tag="proj")
            nc.tensor.matmul(srci_ps[:], w4_sb[:, 1:2], xt[:, hs],
                             start=True, stop=True)
            nc.vector.tensor_copy(srci_sb[:, hs], srci_ps[:])
            dst_ps = pa_proj.tile([2, 512], f32, tag="proj")
            nc.tensor.matmul(dst_ps[:], w4_sb[:, 2:4], xt[:, hs],
                             start=True, stop=True)
            nc.vector.tensor_copy(dst_sb[:, hs], dst_ps[:])
        nc.vector.tensor_scalar(srcr_sb[:], srcr_sb[:], ab_sb[0:1, 0:1],
                                None, op0=Alu.add)
        nc.vector.tensor_scalar(srci_sb[:], srci_sb[:], ab_sb[0:1, 1:2],
                                None, op0=Alu.add)

    with tc.tile_pool(name="pa_bc", bufs=2, space="PSUM") as pa_bc, \
         tc.tile_pool(name="pa_tpd", bufs=2, space="PSUM") as pa_tpd, \
         tc.tile_pool(name="pa_ab", bufs=2, space="PSUM") as pa_ab:
        for row_sb, dstt in ((srcr_sb, src_bc_r), (srci_sb, src_bc_i)):
            for h in range(2):
                hs = slice(h * 512, (h + 1) * 512)
                bc = pa_bc.tile([P, 512], f32, tag="srcbc")
                nc.tensor.matmul(bc[:], ones_row[:], row_sb[:, hs],
                                 start=True, stop=True)
                if h == 0:
                    nc.vector.tensor_copy(dstt[:, hs], bc[:])
                else:
                    nc.vector.tensor_copy(dstt[:, hs], bc[:])

        tp3 = pa_tpd.tile([P, 2 * CH], f32, tag="tpd")
        for c in range(CH):
            nc.tensor.matmul(tp3[:, 2 * c:2 * c + 2],
                             dst_sb[:, c * P:(c + 1) * P], ident[0:2, 0:2],
                             is_transpose=True, start=(c == 0), stop=(c == 7),
                             skip_group_check=True)
        nc.vector.tensor_copy(dstT[:], tp3[:])

        for c in range(CH):
            cs = slice(c * P, (c + 1) * P)
            psA = pa_ab.tile([P, K1 * OC], f32, tag="psAB")
            nc.tensor.matmul(psA[:], xtb[0:C, cs], wcat_sb[0:C, :],
                             start=True, stop=True)
            if c % 2 == 0:
                nc.vector.tensor_copy(As_t[c][:].rearrange("p a b -> p (a b)"), psA[:])
            else:
                nc.vector.tensor_copy(
                    As_t[c][:].rearrange("p a b -> p (a b)"), psA[:])
            psB = pa_ab.tile([P, K1 * OC], f32, tag="psAB")
            nc.tensor.matmul(psB[:], xtb[C:2 * C, cs], wcat_sb[C:2 * C, :],
                             start=True, stop=True)
            if c % 2 == 0:
                nc.vector.tensor_copy(Bs_t[c][:].rearrange("p a b -> p (a b)"), psB[:])
            else:
                nc.vector.tensor_copy(
                    Bs_t[c][:].rearrange("p a b -> p (a b)"), psB[:])

    pa_ctx.close()  # free phase-A SBUF (xr/xi/src/dst staging)

    # ---------------- era 2: attention (per-chunk, depth-2 pipeline) ----
    st_pool = ctx.enter_context(tc.tile_pool(name="stp", bufs=5))
    sq_pool = ctx.enter_context(tc.tile_pool(name="sqp", bufs=2))
    ch_pool = ctx.enter_context(tc.tile_pool(name="chp", bufs=7))
    er_pool = ctx.enter_context(tc.tile_pool(name="erp", bufs=2))
    e_pool = ctx.enter_context(tc.tile_pool(name="epool", bufs=1))

    for c in range(CH):
        sTr = st_pool.tile([P, N], f32, tag="stp", name=f"sTr{c}")
        nc.scalar.activation(sTr[:], src_bc_r[:], Act.Prelu,
                             bias=dstT[:, 2 * c:2 * c + 1],
                             alpha=pa_sb[:, 0:1])
        sTi = st_pool.tile([P, N], f32, tag="stp", name=f"sTi{c}")
        nc.scalar.activation(sTi[:], src_bc_i[:], Act.Prelu,
                             bias=dstT[:, 2 * c + 1:2 * c + 2],
                             alpha=pa_sb[:, 1:2])
        sqr = sq_pool.tile([P, N], f32, tag="sqp", name=f"sqr{c}")
        nc.vector.tensor_tensor(sqr[:], sTr[:], sTr[:], op=Alu.mult)
        sqi = sq_pool.tile([P, N], f32, tag="sqp", name=f"sqi{c}")
        nc.gpsimd.tensor_tensor(sqi[:], sTi[:], sTi[:], op=Alu.mult)
        m2 = ch_pool.tile([P, N], f32, tag="chp", name=f"m2_{c}")
        nc.vector.tensor_tensor(m2[:], sqr[:], sqi[:], op=Alu.add)
        lnm2 = ch_pool.tile([P, N], f32, tag="chp", name=f"ln{c}")
        nc.scalar.activation(lnm2[:], m2[:], Act.Ln, bias=eps_col[:, 0:1])
        mag = ch_pool.tile([P, N], f32, tag="chp", name=f"mg{c}")
        nc.scalar.activation(mag[:], lnm2[:], Act.Exp, scale=0.5)
        e_scr = e_pool.tile([P, N], bf16, tag="epool", name=f"e{c}")
        nc.scalar.activation(e_scr[:], mag[:], Act.Exp,
                             accum_out=cs_cols[:, c:c + 1])
        d = ch_pool.tile([P, N], f32, tag="chp", name=f"d{c}")
        nc.vector.scalar_tensor_tensor(d[:], lnm2[:], -0.5, mag[:],
                                       op0=Alu.mult, op1=Alu.add)
        er = er_pool.tile([P, N], bf16, tag="erp", name=f"er{c}")
        nc.scalar.activation(er[:], d[:], Act.Exp)
        nc.vector.reciprocal(inv_cols[:, c:c + 1], cs_cols[:, c:c + 1])
        nc.vector.scalar_tensor_tensor(ar_t[c][:], sTr[:],
                                       inv_cols[:, c:c + 1], er[:],
                                       op0=Alu.mult, op1=Alu.mult)
        nc.vector.scalar_tensor_tensor(ai_t[c][:], sTi[:],
                                       inv_cols[:, c:c + 1], er[:],
                                       op0=Alu.mult, op1=Alu.mult)
        nc.vector.tensor_scalar(mai_t[c][:], ai_t[c][:], -1.0, None,
                                op0=Alu.mult)

    # ---------------- era 2/3: PE diag passes + extraction + tail -------
    cp_pool = ctx.enter_context(tc.tile_pool(name="cpp", bufs=1))
    stk_pool = ctx.enter_context(tc.tile_pool(name="stkp", bufs=2 * CH))
    o_pool = ctx.enter_context(tc.tile_pool(name="op", bufs=18))

    # stacked per-ib output accumulators: slot (s, k): s0 = As*SL, s1 = Bs*SL
    stk_r = [stk_pool.tile([P, 2, K1 - 1, OC], bf16, tag="stkp",
                           name=f"skr{ib}") for ib in range(CH)]
    stk_i = [stk_pool.tile([P, 2, K1 - 1, OC], bf16, tag="stkp",
                           name=f"ski{ib}") for ib in range(CH)]

    part_rt = []
    with tc.tile_pool(name="diag_ps", bufs=2, space="PSUM") as dps:
        for k in range(K1):
            if k < K1 - 1:
                ps = dps.tile([P, CH, 2, P], f32, tag="dps", name=f"ps{k}")
                for c in range(CH):
                    lt = l_tiles[(k, c)]
                    last = (c == CH - 1)
                    for ib in range(CH):
                        ibs = slice(ib * P, (ib + 1) * P)
                        _diag_mms(nc, ps[:, ib, :, :], ar_t[c], ai_t[c],
                                  mai_t[c], lt, ibs,
                                  (c == 0 and ib % 2 == 0), last)
            else:
                # last pass: ib-outer sweep over one tile, then per-ib tails
                ps = dps.tile([P, CH, 2, P], f32, tag="dps", name=f"ps{k}")
                for ib in range(CH):
                    for c in range(CH):
                        lt = l_tiles[(k, c)]
                        ibs = slice(ib * P, (ib + 1) * P)
                        _diag_mms(nc, ps[:, ib, :, :], ar_t[c], ai_t[c],
                                  mai_t[c], lt, ibs,
                                  (c == 0 and ib % 2 == 0), c == CH - 1)
                for ib in range(CH):
                    _k4_tail(nc, Alu, X, ib, ps[:, ib, :, :], cp_pool,
                             o_pool, mask_bf, slAll, part_rt, As_t, Bs_t,
                             out_r, out_i, P, K1, OC, bf16, f32, k)
            if k < K1 - 1:
                # extraction: DVE mask-stt directly on psum->bf16, then reduce
                mskd = cp_pool.tile([P, CH * 2, P], bf16, tag="cpp",
                                    name=f"mk{k}")
                mask_bc = mask_bf[:].unsqueeze(1).broadcast_to([P, CH * 2, P])
                nc.vector.scalar_tensor_tensor(
                    mskd[:], ps[:].rearrange("p a b c -> p (a b) c"), 1.0,
                    mask_bc, op0=Alu.bypass, op1=Alu.mult)
                nc.vector.reduce_sum(
                    slAll[:, :, :, k].rearrange("p a b -> p (a b)"),
                    mskd[:], axis=X)
                for ib in range(CH):
                    _stk_ops(nc, Alu, Act, ib, k, slAll, stk_r, stk_i,
                             As_t, Bs_t)
                if k == K1 - 2:
                    # partial output reduction over slots (s, k0..k3)
                    for ib in range(CH):
                        orc = o_pool.tile([P, OC], f32, tag="opr",
                                          name=f"or{ib}")
                        oic = o_pool.tile([P, OC], f32, tag="opr",
                                          name=f"oi{ib}")
                        part_rt.append((orc, oic))
                        nc.vector.reduce_sum(
                            orc[:],
                            stk_r[ib][:].rearrange("p s k o -> p o (s k)"),
                            axis=X)
                        nc.vector.reduce_sum(
                            oic[:],
                            stk_i[ib][:].rearrange("p s k o -> p o (s k)"),
                            axis=X)


def _diag_mms(nc, out3, ar, ai, mai, lt, ibs, first, last):
    # mm1: ar x (Lr|Li) -> (t0 += ar*Lr, t1 += ar*Li)
    nc.tensor.matmul(out3, ar[:, ibs], lt[:, :, ibs], start=first,
                     stop=False, skip_group_check=True)
    # mm2: (-ai) x Li -> t0   (=> t0 = sum ar*Lr - ai*Li = SLr)
    nc.tensor.matmul(out3[:, 0, :], mai[:, ibs], lt[:, 1, ibs],
                     start=False, stop=last, skip_group_check=True)
    # mm3: ai x Lr -> t1      (=> t1 = sum ar*Li + ai*Lr = SLi)
    nc.tensor.matmul(out3[:, 1, :], ai[:, ibs], lt[:, 0, ibs],
                     start=False, stop=last, skip_group_check=True)


def _k4_tail(nc, Alu, X, ib, ps2, cp_pool, o_pool, mask_bf, slAll, part_rt,
             As_t, Bs_t, out_r, out_i, P, K1, OC, bf16, f32, k):
    mask_bc2 = mask_bf[:].unsqueeze(1).broadcast_to([P, 2, P])
    mskd = cp_pool.tile([P, 2, P], bf16, tag="cps", name=f"mk4_{ib}")
    nc.vector.scalar_tensor_tensor(mskd[:], ps2, 1.0, mask_bc2,
                                   op0=Alu.bypass, op1=Alu.mult)
    nc.vector.reduce_sum(slAll[:, ib, :, k], mskd[:], axis=X)
    slr4 = slAll[:, ib, 0, k:k + 1]
    sli4 = slAll[:, ib, 1, k:k + 1]
    msli4 = cp_pool.tile([P, 1], f32, tag="ng", name=f"ng{ib}")
    nc.vector.tensor_scalar(msli4[:], sli4, -1.0, None, op0=Alu.mult)
    orc, oic = part_rt[ib]
    or2 = o_pool.tile([P, OC], f32, tag="op2", name=f"o2r{ib}")
    oi2 = o_pool.tile([P, OC], f32, tag="op2", name=f"o2i{ib}")
    # final = partial + As*SLr4 - Bs*SLi4 (fused stt chain)
    nc.vector.scalar_tensor_tensor(or2[:], As_t[ib][:, K1 - 1, :], slr4,
                                   orc[:], op0=Alu.mult, op1=Alu.add)
    nc.vector.scalar_tensor_tensor(or2[:], Bs_t[ib][:, K1 - 1, :],
                                   msli4[:, 0:1], or2[:],
                                   op0=Alu.mult, op1=Alu.add)
    nc.vector.scalar_tensor_tensor(oi2[:], As_t[ib][:, K1 - 1, :], sli4,
                                   oic[:], op0=Alu.mult, op1=Alu.add)
    nc.vector.scalar_tensor_tensor(oi2[:], Bs_t[ib][:, K1 - 1, :], slr4,
                                   oi2[:], op0=Alu.mult, op1=Alu.add)
    nc.sync.dma_start(out_r[ib * P:(ib + 1) * P, :], or2[:])
    nc.sync.dma_start(out_i[ib * P:(ib + 1) * P, :], oi2[:])


def _stk_ops(nc, Alu, Act, ib, k, slAll, stk_r, stk_i, As_t, Bs_t):
    """stk slots for pass k: s0 = A*SL, s1 = B*(+-SL). The r-ops run on DVE
    (ts 4x mode bf16); the i-ops on ACT as scale-Copy (ACT is idle here)."""
    slr = slAll[:, ib, 0, k:k + 1]
    sli = slAll[:, ib, 1, k:k + 1]
    nc.vector.tensor_scalar(stk_r[ib][:, 0, k, :], As_t[ib][:, k, :],
                            slr, None, op0=Alu.mult)
    nc.vector.tensor_scalar(stk_r[ib][:, 1, k, :], Bs_t[ib][:, k, :],
                            sli, -1.0, op0=Alu.mult, op1=Alu.mult)
    nc.scalar.activation(stk_i[ib][:, 0, k, :], As_t[ib][:, k, :],
                         Act.Copy, scale=sli)
    nc.scalar.activation(stk_i[ib][:, 1, k, :], Bs_t[ib][:, k, :],
                         Act.Copy, scale=slr)


def _host_prep(inputs):
    import ml_dtypes
    BF = ml_dtypes.bfloat16
    f = lambda k: np.ascontiguousarray(np.asarray(inputs[k], dtype=np.float32))
    X_real, X_imag = f("X_real"), f("X_imag")
    w_real, w_imag = f("w_real"), f("w_imag")
    aw_real, aw_imag = f("aw_real"), f("aw_imag")
    ab_real = float(np.asarray(inputs["ab_real"]))
    ab_imag = float(np.asarray(inputs["ab_imag"]))
    pa_real = float(np.asarray(inputs["pa_real"]))
    pa_imag = float(np.asarray(inputs["pa_imag"]))

    # lcat[b]: [k][c][j 128][t 2][i 1024] with t0 = Lr^T, t1 = Li^T
    Lr = np.asarray(inputs["L_real"], dtype=np.float32)
    Li = np.asarray(inputs["L_imag"], dtype=np.float32)
    LrT = Lr.transpose(0, 1, 3, 2).reshape(B, K1, CH, P, N)
    LiT = Li.transpose(0, 1, 3, 2).reshape(B, K1, CH, P, N)
    lcat = np.ascontiguousarray(
        np.stack([LrT, LiT], axis=4).astype(BF))     # [B, K1, CH, P, 2, N]

    ws_r, wd_r = aw_real[:C], aw_real[C:]
    ws_i, wd_i = aw_imag[:C], aw_imag[C:]
    w4 = np.stack([
        np.concatenate([ws_r, -ws_i]),
        np.concatenate([ws_i, ws_r]),
        np.concatenate([wd_r, -wd_i]),
        np.concatenate([wd_i, wd_r]),
    ], axis=1).astype(np.float32)

    wr_t = w_real.transpose(1, 0, 2).reshape(C, K1 * OC)
    wi_t = w_imag.transpose(1, 0, 2).reshape(C, K1 * OC)
    wcat = np.concatenate([wr_t, wi_t], axis=0).astype(BF)

    pa_cols = np.stack([np.full(P, pa_real), np.full(P, pa_imag)],
                       axis=1).astype(np.float32)
    ab2 = np.array([[ab_real, ab_imag]], dtype=np.float32)

    in_maps = []
    for b in range(NCORES):
        xr_p = np.ascontiguousarray(
            X_real[b].reshape(CH, P, C).transpose(1, 0, 2))
        xi_p = np.ascontiguousarray(
            X_imag[b].reshape(CH, P, C).transpose(1, 0, 2))
        in_maps.append({
            "x_real": xr_p, "x_imag": xi_p,
            "lcat": lcat[b],
            "w4": w4, "wcat": wcat,
            "pa_cols": pa_cols, "ab2": ab2,
        })
    return in_maps


def kernel(**inputs):
    import os
    from concourse import bass_utils

    if "nc" not in _CACHE:
        _CACHE["nc"] = _build_nc()
    nc = _CACHE["nc"]
    in_maps = _host_prep(inputs)
    trace = os.environ.get("KERNEL_TRACE", "0") == "1"
    res = bass_utils.run_bass_kernel_spmd(nc, in_maps,
                                          core_ids=list(range(NCORES)),
                                          trace=trace)
    _CACHE["last_result"] = res
    out_r = np.stack([res.results[b]["out_r"] for b in range(NCORES)])
    out_i = np.stack([res.results[b]["out_i"] for b in range(NCORES)])
    return out_r, out_i


# revision 38
# speedup vs baseline: 1.0063x; 1.0063x over previous
"""Trainium2 Bass kernel for nn_ChebNet (complex Chebyshev GNN layer).

Sharding: data-parallel over batch B=8 across the 8 NeuronCores (one batch
element per core). No inter-core communication.

v3 design (vs 249us v2): full phase overlap + engine balance.

Structure (per core, batch b; N=1024, C=OC=64, K+1=5, j-chunks CH=8):
  era1: DMA weights/X; PE transposes X^T, attention projections, src
    broadcast rows, dst cols, A/B output stacks (bf16). L (bf16,
    host-pretiled [k][c][j][t][i]) prefetch starts at t=0.
  era2: attention per j-chunk in TRANSPOSED layout [j, i] (softmax over i
    = free axis): ACT chain Prelu x2 -> (Pool squares, Pool m2-add) ->
    Ln -> Exp(0.5 ln) = mag -> Exp(mag)+accum colsum -> er = Exp(mag -
    0.5 lnm2); DVE: d-stt, ar/ai stt (bf16), mai. Ln/mag/er ops merged
    over chunk PAIRS. One act table set (natural_log_exp) enforced by a
    post-compile fixup (the stock pass thrashes 0<->5 per chunk).
    Meanwhile PE runs diag-product passes k0,k1 gated per chunk.
  era3: PE passes k2..k4 at full clock. Per k-pass PSUM [128, 8ib, 2t,
    128] (4 banks, 2 passes in flight): per (c, ib) 3 matmuls:
    ar x (Lr|Li) -> (t0,t1); mai x Li -> t0; ai x Lr -> t1, so psum
    t0 = sum_j ar*Lr - ai*Li (= SLr diag block) and t1 = SLi. One
    start=True per bank (c==0, even ib) resets it; no zero-matmuls.
    Extraction per pass: ACT copy psum->bf16, DVE mask-mult (2x bf16),
    DVE reduce -> slAll[128, ib, t, k] strided. Tail per ib: Pool/DVE
    broadcast-tt Z = SL (x) A/B stacks -> strided reduce -> out DMA.
"""

import numpy as np
from contextlib import ExitStack

B, N, C, OC = 8, 1024, 64, 64
K1 = 5
P = 128
CH = N // P
NCORES = 8
EPS = 1e-12

_CACHE = {}


def _build_nc():
    import concourse.tile as tile
    from concourse import bacc, mybir

    f32 = mybir.dt.float32
    bf16 = mybir.dt.bfloat16
    Alu = mybir.AluOpType
    Act = mybir.ActivationFunctionType

    nc = bacc.Bacc("TRN2", target_bir_lowering=False, debug=False,
                   enable_asserts=False, num_devices=NCORES)

    # host-pretiled to [p 128][c 8][d 64] so the DMA is contiguous
    x_real = nc.dram_tensor("x_real", [P, CH, C], f32, kind="ExternalInput").ap()
    x_imag = nc.dram_tensor("x_imag", [P, CH, C], f32, kind="ExternalInput").ap()
    # host-pretiled: [k][c][j 128][t 2][i 1024], t0=Lr^T, t1=Li^T (bf16)
    lcat = nc.dram_tensor("lcat", [K1, CH, P, 2, N], bf16,
                          kind="ExternalInput").ap()
    w4 = nc.dram_tensor("w4", [2 * C, 4], f32, kind="ExternalInput").ap()
    wcat = nc.dram_tensor("wcat", [2 * C, K1 * OC], bf16,
                          kind="ExternalInput").ap()
    pa_cols = nc.dram_tensor("pa_cols", [P, 2], f32, kind="ExternalInput").ap()
    ab2 = nc.dram_tensor("ab2", [1, 2], f32, kind="ExternalInput").ap()
    out_r = nc.dram_tensor("out_r", [N, OC], f32, kind="ExternalOutput").ap()
    out_i = nc.dram_tensor("out_i", [N, OC], f32, kind="ExternalOutput").ap()

    with tile.TileContext(nc) as tc:
        with ExitStack() as ctx:
            _emit(ctx, tc, nc, mybir, f32, bf16, Alu, Act,
                  x_real, x_imag, lcat, w4, wcat, pa_cols, ab2, out_r, out_i)
    nc.compile()

    # --- act-table fixup: the stock placement alternates between table
    # sets 0 (exp) and 5 (ln) every chunk (~1.3us per reload). Set 6
    # (natural_log_exp_and_others) serves every function used here
    # (parametric_relu, square, ln, exp, copy), so keep one load of it.
    nloads = 0
    for b in nc.main_func.blocks:
        keep = []
        for inst in b.instructions:
            if isinstance(inst, mybir.InstLoadActFuncSet):
                nloads += 1
                if nloads == 1:
                    inst.act_func_set_id = 6
                    keep.append(inst)
                continue
            keep.append(inst)
        b.instructions[:] = keep
    return nc


def _emit(ctx, tc, nc, mybir, f32, bf16, Alu, Act,
          x_real, x_imag, lcat, w4, wcat, pa_cols, ab2, out_r, out_i):
    X = mybir.AxisListType.X

    const = ctx.enter_context(tc.tile_pool(name="const", bufs=1))
    persist = ctx.enter_context(tc.tile_pool(name="persist", bufs=1))

    ident_i = const.tile([P, P], mybir.dt.int32)
    nc.gpsimd.iota(ident_i[:], pattern=[[1, P]], base=0, channel_multiplier=-1)
    ident = const.tile([P, P], f32)
    nc.vector.tensor_scalar(ident[:], ident_i[:], 0, None, op0=Alu.is_equal)
    mask_bf = const.tile([P, P], bf16)
    nc.vector.tensor_scalar(mask_bf[:], ident_i[:], 0, None, op0=Alu.is_equal)

    ones_row = const.tile([1, P], f32)
    nc.vector.memset(ones_row[:], 1.0)
    eps_col = const.tile([P, 1], f32)
    nc.vector.memset(eps_col[:], EPS)

    w4_sb = const.tile([P, 4], f32)
    nc.sync.dma_start(w4_sb[:], w4[:])
    wcat_sb = const.tile([P, K1 * OC], bf16)
    nc.sync.dma_start(wcat_sb[:], wcat[:])
    pa_sb = const.tile([P, 2], f32)
    nc.sync.dma_start(pa_sb[:], pa_cols[:])
    ab_sb = const.tile([1, 2], f32)
    nc.sync.dma_start(ab_sb[:], ab2[:])

    # ---------------- persistent tiles ----------------
    src_bc_r = persist.tile([P, N], f32)
    src_bc_i = persist.tile([P, N], f32)
    dstT = persist.tile([P, 2 * CH], f32)

    a_pool = ctx.enter_context(tc.tile_pool(name="apool", bufs=3 * CH))
    ar_t, ai_t, mai_t = [], [], []
    for c in range(CH):
        ar_t.append(a_pool.tile([P, N], bf16, tag="apool", name=f"ar{c}"))
        ai_t.append(a_pool.tile([P, N], bf16, tag="apool", name=f"ai{c}"))
        mai_t.append(a_pool.tile([P, N], bf16, tag="apool", name=f"mai{c}"))

    ab_stack = ctx.enter_context(tc.tile_pool(name="abst", bufs=2 * CH))
    As_t, Bs_t = [], []
    for c in range(CH):
        As_t.append(ab_stack.tile([P, K1, OC], bf16, tag="abst", name=f"As{c}"))
        Bs_t.append(ab_stack.tile([P, K1, OC], bf16, tag="abst", name=f"Bs{c}"))

    # L tiles: rotating pool, one tile per (k, c) = [128j, 2t, 1024i] bf16
    # (DMAs emitted after the phase-A loads so X isn't queued behind 20MB)
    l_pool = ctx.enter_context(tc.tile_pool(name="lpool", bufs=10))
    l_tiles = {}

    # SL results [p, ib, t(R/I), k] fp32
    slAll = persist.tile([P, CH, 2, K1], f32)
    cs_cols = persist.tile([P, CH], f32)     # per-chunk softmax colsums
    inv_cols = persist.tile([P, CH], f32)

    # ---------------- era 1: transposes / projections / stacks ----------
    pa_ctx = ExitStack()
    pa_pool = pa_ctx.enter_context(tc.tile_pool(name="phaseA", bufs=1))
    with tc.tile_pool(name="pa_ps", bufs=2, space="PSUM") as pa_ps, \
         tc.tile_pool(name="pa_proj", bufs=3, space="PSUM") as pa_proj:
        xt = pa_pool.tile([P, N], f32)         # [Xr^T; Xi^T]
        xtb = pa_pool.tile([P, N], bf16)
        xr_sb = pa_pool.tile([P, CH, C], f32)
        xi_sb = pa_pool.tile([P, CH, C], f32)
        for q in range(4):
            qs = slice(q * 2, q * 2 + 2)
            nc.sync.dma_start(xr_sb[:, qs, :], x_real[:, qs, :])
            nc.sync.dma_start(xi_sb[:, qs, :], x_imag[:, qs, :])

        # L prefetch: emitted now (after X/weight loads) so it starts at
        # t~0 without delaying the phase-A inputs.
        for k in range(K1):
            for c in range(CH):
                lt = l_pool.tile([P, 2, N], bf16, tag="lpool",
                                 name=f"L{k}_{c}")
                l_tiles[(k, c)] = lt
                nc.sync.dma_start(lt[:], lcat[k, c])

        # all 8 transposes per r/i go into one psum tile -> ONE copy each.
        # tile is 2 banks (1024 f32); start=True resets a whole bank, so
        # flag it only on the first write of each bank (c==0, c==4).
        tpr = pa_ps.tile([C, CH, P], f32, tag="tp")
        for c in range(CH):
            nc.tensor.matmul(tpr[:, c, :], xr_sb[:, c, :], ident[:],
                             is_transpose=True, start=(c in (0, 4)),
                             stop=(c in (3, 7)), skip_group_check=True)
            if c == 3:
                nc.vector.tensor_copy(
                    xt[0:C, 0:512], tpr[:, 0:4, :].rearrange("p a b -> p (a b)"))
        nc.vector.tensor_copy(
            xt[0:C, 512:1024], tpr[:, 4:8, :].rearrange("p a b -> p (a b)"))
        tpi = pa_ps.tile([C, CH, P], f32, tag="tp")
        for c in range(CH):
            nc.tensor.matmul(tpi[:, c, :], xi_sb[:, c, :], ident[:],
                             is_transpose=True, start=(c in (0, 4)),
                             stop=(c in (3, 7)), skip_group_check=True)
            if c == 3:
                nc.scalar.copy(
                    xt[C:2 * C, 0:512],
                    tpi[:, 0:4, :].rearrange("p a b -> p (a b)"))
        nc.scalar.copy(xt[C:2 * C, 512:1024],
                       tpi[:, 4:8, :].rearrange("p a b -> p (a b)"))

        nc.vector.tensor_copy(xtb[:], xt[:])

        srcr_sb = pa_pool.tile([1, N], f32)
        srci_sb = pa_pool.tile([1, N], f32)
        dst_sb = pa_pool.tile([2, N], f32)
        for h in range(2):
            hs = slice(h * 512, (h + 1) * 512)
            srcr_ps = pa_proj.tile([1, 512], f32, tag="proj")
            nc.tensor.matmul(srcr_ps[:], w4_sb[:, 0:1], xt[:, hs],
                             start=True, stop=True)
            nc.scalar.copy(srcr_sb[:, hs], srcr_ps[:])
            srci_ps = pa_proj.tile([1, 512], f32, tag="proj")
            nc.tensor.matmul(srci_ps[:], w4_sb[:, 1:2], xt[:, hs],
                             start=True, stop=True)
            nc.scalar.copy(srci_sb[:, hs], srci_ps[:])
            dst_ps = pa_proj.tile([2, 512], f32, tag="proj")
            nc.tensor.matmul(dst_ps[:], w4_sb[:, 2:4], xt[:, hs],
                             start=True, stop=True)
            nc.scalar.copy(dst_sb[:, hs], dst_ps[:])
        nc.vector.tensor_scalar(srcr_sb[:], srcr_sb[:], ab_sb[0:1, 0:1],
                                None, op0=Alu.add)
        nc.vector.tensor_scalar(srci_sb[:], srci_sb[:], ab_sb[0:1, 1:2],
                                None, op0=Alu.add)

    with tc.tile_pool(name="pa_bc", bufs=2, space="PSUM") as pa_bc, \
         tc.tile_pool(name="pa_tpd", bufs=2, space="PSUM") as pa_tpd, \
         tc.tile_pool(name="pa_ab", bufs=2, space="PSUM") as pa_ab:
        for row_sb, dstt in ((srcr_sb, src_bc_r), (srci_sb, src_bc_i)):
            for h in range(2):
                hs = slice(h * 512, (h + 1) * 512)
                bc = pa_bc.tile([P, 512], f32, tag="srcbc")
                nc.tensor.matmul(bc[:], ones_row[:], row_sb[:, hs],
                                 start=True, stop=True)
                if h == 0:
                    nc.vector.tensor_copy(dstt[:, hs], bc[:])
                else:
                    nc.scalar.copy(dstt[:, hs], bc[:])

        tp3 = pa_tpd.tile([P, 2 * CH], f32, tag="tpd")
        for c in range(CH):
            nc.tensor.matmul(tp3[:, 2 * c:2 * c + 2],
                             dst_sb[:, c * P:(c + 1) * P], ident[0:2, 0:2],
                             is_transpose=True, start=(c == 0), stop=(c == 7),
                             skip_group_check=True)
        nc.vector.tensor_copy(dstT[:], tp3[:])

        for c in range(CH):
            cs = slice(c * P, (c + 1) * P)
            psA = pa_ab.tile([P, K1 * OC], f32, tag="psAB")
            nc.tensor.matmul(psA[:], xtb[0:C, cs], wcat_sb[0:C, :],
                             start=True, stop=True)
            if c % 2 == 0:
                nc.scalar.copy(As_t[c][:].rearrange("p a b -> p (a b)"), psA[:])
            else:
                nc.vector.tensor_copy(
                    As_t[c][:].rearrange("p a b -> p (a b)"), psA[:])
            psB = pa_ab.tile([P, K1 * OC], f32, tag="psAB")
            nc.tensor.matmul(psB[:], xtb[C:2 * C, cs], wcat_sb[C:2 * C, :],
                             start=True, stop=True)
            if c % 2 == 0:
                nc.scalar.copy(Bs_t[c][:].rearrange("p a b -> p (a b)"), psB[:])
            else:
                nc.vector.tensor_copy(
                    Bs_t[c][:].rearrange("p a b -> p (a b)"), psB[:])

    pa_ctx.close()  # free phase-A SBUF (xr/xi/src/dst staging)

    # ---------------- era 2: attention (per-chunk, depth-2 pipeline) ----
    st_pool = ctx.enter_context(tc.tile_pool(name="stp", bufs=5))
    sq_pool = ctx.enter_context(tc.tile_pool(name="sqp", bufs=2))
    ch_pool = ctx.enter_context(tc.tile_pool(name="chp", bufs=7))
    er_pool = ctx.enter_context(tc.tile_pool(name="erp", bufs=2))
    e_pool = ctx.enter_context(tc.tile_pool(name="epool", bufs=1))

    for c in range(CH):
        sTr = st_pool.tile([P, N], f32, tag="stp", name=f"sTr{c}")
        nc.scalar.activation(sTr[:], src_bc_r[:], Act.Prelu,
                             bias=dstT[:, 2 * c:2 * c + 1],
                             alpha=pa_sb[:, 0:1])
        sTi = st_pool.tile([P, N], f32, tag="stp", name=f"sTi{c}")
        nc.scalar.activation(sTi[:], src_bc_i[:], Act.Prelu,
                             bias=dstT[:, 2 * c + 1:2 * c + 2],
                             alpha=pa_sb[:, 1:2])
        sqr = sq_pool.tile([P, N], f32, tag="sqp", name=f"sqr{c}")
        nc.vector.tensor_tensor(sqr[:], sTr[:], sTr[:], op=Alu.mult)
        sqi = sq_pool.tile([P, N], f32, tag="sqp", name=f"sqi{c}")
        nc.gpsimd.tensor_tensor(sqi[:], sTi[:], sTi[:], op=Alu.mult)
        m2 = ch_pool.tile([P, N], f32, tag="chp", name=f"m2_{c}")
        nc.vector.tensor_tensor(m2[:], sqr[:], sqi[:], op=Alu.add)
        lnm2 = ch_pool.tile([P, N], f32, tag="chp", name=f"ln{c}")
        nc.scalar.activation(lnm2[:], m2[:], Act.Ln, bias=eps_col[:, 0:1])
        mag = ch_pool.tile([P, N], f32, tag="chp", name=f"mg{c}")
        nc.scalar.activation(mag[:], lnm2[:], Act.Exp, scale=0.5)
        e_scr = e_pool.tile([P, N], bf16, tag="epool", name=f"e{c}")
        nc.scalar.activation(e_scr[:], mag[:], Act.Exp,
                             accum_out=cs_cols[:, c:c + 1])
        d = ch_pool.tile([P, N], f32, tag="chp", name=f"d{c}")
        nc.vector.scalar_tensor_tensor(d[:], lnm2[:], -0.5, mag[:],
                                       op0=Alu.mult, op1=Alu.add)
        er = er_pool.tile([P, N], bf16, tag="erp", name=f"er{c}")
        nc.scalar.activation(er[:], d[:], Act.Exp)
        nc.vector.reciprocal(inv_cols[:, c:c + 1], cs_cols[:, c:c + 1])
        nc.vector.scalar_tensor_tensor(ar_t[c][:], sTr[:],
                                       inv_cols[:, c:c + 1], er[:],
                                       op0=Alu.mult, op1=Alu.mult)
        nc.vector.scalar_tensor_tensor(ai_t[c][:], sTi[:],
                                       inv_cols[:, c:c + 1], er[:],
                                       op0=Alu.mult, op1=Alu.mult)
        nc.vector.tensor_scalar(mai_t[c][:], ai_t[c][:], -1.0, None,
                                op0=Alu.mult)

    # ---------------- era 2/3: PE diag passes + extraction + tail -------
    cp_pool = ctx.enter_context(tc.tile_pool(name="cpp", bufs=1))
    stk_pool = ctx.enter_context(tc.tile_pool(name="stkp", bufs=2 * CH))
    o_pool = ctx.enter_context(tc.tile_pool(name="op", bufs=18))

    # stacked per-ib output accumulators: slot (s, k): s0 = As*SL, s1 = Bs*SL
    stk_r = [stk_pool.tile([P, 2, K1 - 1, OC], bf16, tag="stkp",
                           name=f"skr{ib}") for ib in range(CH)]
    stk_i = [stk_pool.tile([P, 2, K1 - 1, OC], bf16, tag="stkp",
                           name=f"ski{ib}") for ib in range(CH)]

    part_rt = []
    with tc.tile_pool(name="diag_ps", bufs=2, space="PSUM") as dps:
        for k in range(K1):
            if k < K1 - 1:
                ps = dps.tile([P, CH, 2, P], f32, tag="dps", name=f"ps{k}")
                for c in range(CH):
                    lt = l_tiles[(k, c)]
                    last = (c == CH - 1)
                    for ib in range(CH):
                        ibs = slice(ib * P, (ib + 1) * P)
                        _diag_mms(nc, ps[:, ib, :, :], ar_t[c], ai_t[c],
                                  mai_t[c], lt, ibs,
                                  (c == 0 and ib % 2 == 0), last)
            else:
                # last pass: ib-outer sweep over one tile, then per-ib tails
                ps = dps.tile([P, CH, 2, P], f32, tag="dps", name=f"ps{k}")
                for ib in range(CH):
                    for c in range(CH):
                        lt = l_tiles[(k, c)]
                        ibs = slice(ib * P, (ib + 1) * P)
                        _diag_mms(nc, ps[:, ib, :, :], ar_t[c], ai_t[c],
                                  mai_t[c], lt, ibs,
                                  (c == 0 and ib % 2 == 0), c == CH - 1)
                for ib in range(CH):
                    _k4_tail(nc, Alu, X, ib, ps[:, ib, :, :], cp_pool,
                             o_pool, mask_bf, slAll, part_rt, As_t, Bs_t,
                             out_r, out_i, P, K1, OC, bf16, f32, k)
            if k < K1 - 1:
                # extraction: DVE mask-stt directly on psum->bf16, then reduce
                mskd = cp_pool.tile([P, CH * 2, P], bf16, tag="cpp",
                                    name=f"mk{k}")
                mask_bc = mask_bf[:].unsqueeze(1).broadcast_to([P, CH * 2, P])
                nc.vector.scalar_tensor_tensor(
                    mskd[:], ps[:].rearrange("p a b c -> p (a b) c"), 1.0,
                    mask_bc, op0=Alu.bypass, op1=Alu.mult)
                nc.vector.reduce_sum(
                    slAll[:, :, :, k].rearrange("p a b -> p (a b)"),
                    mskd[:], axis=X)
                for ib in range(CH):
                    _stk_ops(nc, Alu, Act, ib, k, slAll, stk_r, stk_i,
                             As_t, Bs_t)
                if k == K1 - 2:
                    # partial output reduction over slots (s, k0..k3)
                    for ib in range(CH):
                        orc = o_pool.tile([P, OC], f32, tag="opr",
                                          name=f"or{ib}")
                        oic = o_pool.tile([P, OC], f32, tag="opr",
                                          name=f"oi{ib}")
                        part_rt.append((orc, oic))
                        nc.vector.reduce_sum(
                            orc[:],
                            stk_r[ib][:].rearrange("p s k o -> p o (s k)"),
                            axis=X)
                        nc.vector.reduce_sum(
                            oic[:],
                            stk_i[ib][:].rearrange("p s k o -> p o (s k)"),
                            axis=X)


def _diag_mms(nc, out3, ar, ai, mai, lt, ibs, first, last):
    # mm1: ar x (Lr|Li) -> (t0 += ar*Lr, t1 += ar*Li)
    nc.tensor.matmul(out3, ar[:, ibs], lt[:, :, ibs], start=first,
                     stop=False, skip_group_check=True)
    # mm2: (-ai) x Li -> t0   (=> t0 = sum ar*Lr - ai*Li = SLr)
    nc.tensor.matmul(out3[:, 0, :], mai[:, ibs], lt[:, 1, ibs],
                     start=False, stop=last, skip_group_check=True)
    # mm3: ai x Lr -> t1      (=> t1 = sum ar*Li + ai*Lr = SLi)
    nc.tensor.matmul(out3[:, 1, :], ai[:, ibs], lt[:, 0, ibs],
                     start=False, stop=last, skip_group_check=True)


def _k4_tail(nc, Alu, X, ib, ps2, cp_pool, o_pool, mask_bf, slAll, part_rt,
             As_t, Bs_t, out_r, out_i, P, K1, OC, bf16, f32, k):
    mask_bc2 = mask_bf[:].unsqueeze(1).broadcast_to([P, 2, P])
    mskd = cp_pool.tile([P, 2, P], bf16, tag="cps", name=f"mk4_{ib}")
    nc.vector.scalar_tensor_tensor(mskd[:], ps2, 1.0, mask_bc2,
                                   op0=Alu.bypass, op1=Alu.mult)
    nc.vector.reduce_sum(slAll[:, ib, :, k], mskd[:], axis=X)
    slr4 = slAll[:, ib, 0, k:k + 1]
    sli4 = slAll[:, ib, 1, k:k + 1]
    msli4 = cp_pool.tile([P, 1], f32, tag="ng", name=f"ng{ib}")
    nc.vector.tensor_scalar(msli4[:], sli4, -1.0, None, op0=Alu.mult)
    orc, oic = part_rt[ib]
    or2 = o_pool.tile([P, OC], f32, tag="op2", name=f"o2r{ib}")
    oi2 = o_pool.tile([P, OC], f32, tag="op2", name=f"o2i{ib}")
    # final = partial + As*SLr4 - Bs*SLi4 (fused stt chain)
    nc.vector.scalar_tensor_tensor(or2[:], As_t[ib][:, K1 - 1, :], slr4,
                                   orc[:], op0=Alu.mult, op1=Alu.add)
    nc.vector.scalar_tensor_tensor(or2[:], Bs_t[ib][:, K1 - 1, :],
                                   msli4[:, 0:1], or2[:],
                                   op0=Alu.mult, op1=Alu.add)
    nc.vector.scalar_tensor_tensor(oi2[:], As_t[ib][:, K1 - 1, :], sli4,
                                   oic[:], op0=Alu.mult, op1=Alu.add)
    nc.vector.scalar_tensor_tensor(oi2[:], Bs_t[ib][:, K1 - 1, :], slr4,
                                   oi2[:], op0=Alu.mult, op1=Alu.add)
    nc.sync.dma_start(out_r[ib * P:(ib + 1) * P, :], or2[:])
    nc.sync.dma_start(out_i[ib * P:(ib + 1) * P, :], oi2[:])


def _stk_ops(nc, Alu, Act, ib, k, slAll, stk_r, stk_i, As_t, Bs_t):
    """stk slots for pass k: s0 = A*SL, s1 = B*(+-SL). The r-ops run on DVE
    (ts 4x mode bf16); the i-ops on ACT as scale-Copy (ACT is idle here)."""
    slr = slAll[:, ib, 0, k:k + 1]
    sli = slAll[:, ib, 1, k:k + 1]
    nc.vector.tensor_scalar(stk_r[ib][:, 0, k, :], As_t[ib][:, k, :],
                            slr, None, op0=Alu.mult)
    nc.vector.tensor_scalar(stk_r[ib][:, 1, k, :], Bs_t[ib][:, k, :],
                            sli, -1.0, op0=Alu.mult, op1=Alu.mult)
    nc.scalar.activation(stk_i[ib][:, 0, k, :], As_t[ib][:, k, :],
                         Act.Copy, scale=sli)
    nc.scalar.activation(stk_i[ib][:, 1, k, :], Bs_t[ib][:, k, :],
                         Act.Copy, scale=slr)


def _host_prep(inputs):
    import ml_dtypes
    BF = ml_dtypes.bfloat16
    f = lambda k: np.ascontiguousarray(np.asarray(inputs[k], dtype=np.float32))
    X_real, X_imag = f("X_real"), f("X_imag")
    w_real, w_imag = f("w_real"), f("w_imag")
    aw_real, aw_imag = f("aw_real"), f("aw_imag")
    ab_real = float(np.asarray(inputs["ab_real"]))
    ab_imag = float(np.asarray(inputs["ab_imag"]))
    pa_real = float(np.asarray(inputs["pa_real"]))
    pa_imag = float(np.asarray(inputs["pa_imag"]))

    # lcat[b]: [k][c][j 128][t 2][i 1024] with t0 = Lr^T, t1 = Li^T
    Lr = np.asarray(inputs["L_real"], dtype=np.float32)
    Li = np.asarray(inputs["L_imag"], dtype=np.float32)
    LrT = Lr.transpose(0, 1, 3, 2).reshape(B, K1, CH, P, N)
    LiT = Li.transpose(0, 1, 3, 2).reshape(B, K1, CH, P, N)
    lcat = np.ascontiguousarray(
        np.stack([LrT, LiT], axis=4).astype(BF))     # [B, K1, CH, P, 2, N]

    ws_r, wd_r = aw_real[:C], aw_real[C:]
    ws_i, wd_i = aw_imag[:C], aw_imag[C:]
    w4 = np.stack([
        np.concatenate([ws_r, -ws_i]),
        np.concatenate([ws_i, ws_r]),
        np.concatenate([wd_r, -wd_i]),
        np.concatenate([wd_i, wd_r]),
    ], axis=1).astype(np.float32)

    wr_t = w_real.transpose(1, 0, 2).reshape(C, K1 * OC)
    wi_t = w_imag.transpose(1, 0, 2).reshape(C, K1 * OC)
    wcat = np.concatenate([wr_t, wi_t], axis=0).astype(BF)

    pa_cols = np.stack([np.full(P, pa_real), np.full(P, pa_imag)],
                       axis=1).astype(np.float32)
    ab2 = np.array([[ab_real, ab_imag]], dtype=np.float32)

    in_maps = []
    for b in range(NCORES):
        xr_p = np.ascontiguousarray(
            X_real[b].reshape(CH, P, C).transpose(1, 0, 2))
        xi_p = np.ascontiguousarray(
            X_imag[b].reshape(CH, P, C).transpose(1, 0, 2))
        in_maps.append({
            "x_real": xr_p, "x_imag": xi_p,
            "lcat": lcat[b],
            "w4": w4, "wcat": wcat,
            "pa_cols": pa_cols, "ab2": ab2,
        })
    return in_maps


def kernel(**inputs):
    import os
    from concourse import bass_utils

    if "nc" not in _CACHE:
        _CACHE["nc"] = _build_nc()
    nc = _CACHE["nc"]
    in_maps = _host_prep(inputs)
    trace = os.environ.get("KERNEL_TRACE", "0") == "1"
    res = bass_utils.run_bass_kernel_spmd(nc, in_maps,
                                          core_ids=list(range(NCORES)),
                                          trace=trace)
    _CACHE["last_result"] = res
    out_r = np.stack([res.results[b]["out_r"] for b in range(NCORES)])
    out_i = np.stack([res.results[b]["out_i"] for b in range(NCORES)])
    return out_r, out_i
